# revision 3
# baseline (speedup 1.0000x reference)
"""Bass/Trainium2 kernel for nn_AlternativeSelfAttention (dense transformer).

Shapes: N=4, S=1024, E=1024, H=16, D=64.  8 NeuronCores.

Sharding (hardcoded): core c handles batch n = c//2 and query rows
[ (c%2)*512 , (c%2)*512+512 ) of that batch, for ALL 16 heads.  Each core
needs keys[n]/values[n] fully, its query slice, and all weights.  No
collectives; each core writes a disjoint [512, 1024] slice of the output.

Math (per core, per head h):
    A   = Wq.T @ Wk                      (64x64, tiny)
    Qp  = Xq_h @ A                       (so E_h = Qp_h @ Xk_h.T == q @ k.T)
    P   = exp(E_h / 32)                  (no max-subtraction; |E/32| < ~1.5)
    C_h = P_h @ Xv_h ; denom = P_h.sum(k)   (denom via 64 ones-columns in the
                                             PV stationary -> replicated rows)
    O_h = (C_h / denom) @ Wv.T
    out = concat_h(O_h) @ Wu.T + bu

Everything on the TensorEngine runs in bf16 (1 cyc/row); accumulation f32.
"""

import sys

sys.path.insert(0, "/opt/trn_rl_repo")

import numpy as np

import concourse.bass as bass
import concourse.mybir as mybir
import concourse.tile as tile
from concourse import bacc
from concourse.bass_utils import run_bass_kernel_spmd
from concourse.masks import make_identity

F32 = mybir.dt.float32
BF16 = mybir.dt.bfloat16
AF = mybir.ActivationFunctionType
ALU = mybir.AluOpType

S = 1024          # keys/values sequence length
Q = 512           # queries per core
E = 1024          # embed
H = 16            # heads
D = 64            # head dim
KC = S // 128     # 8 key chunks
EC = E // 128     # 8 embed chunks
QC = Q // 128     # 4 query-row chunks
SCALE = 1.0 / 32.0  # 1/sqrt(E)

# energy jobs: k-chunks grouped 3/3/2 so one job = 3 PSUM banks and the
# PSUM budget (2x3 energy + 1 C' + 1 small = 8 banks) fits exactly.
JOB_CHUNKS = ((0, 1, 2), (3, 4, 5), (6, 7))
CHUNK2JOB = {c: ((c // 3, c % 3) if c < 6 else (2, c - 6)) for c in range(KC)}


def _body(nc, tc, xq, xk, xv, wq, wk, wv, wu, bu, out):
    with (
        tc.tile_pool(name="pp", bufs=1) as pp,
        tc.tile_pool(name="ptp", bufs=6) as ptp,
        tc.tile_pool(name="cnp", bufs=2) as cnp,
        tc.tile_pool(name="rcp", bufs=2) as rcp,
        tc.tile_pool(name="ep", bufs=2, space="PSUM") as ep,
        tc.tile_pool(name="cp", bufs=1, space="PSUM") as cp,
        tc.tile_pool(name="sp", bufs=1, space="PSUM") as sp,
    ):
        # ---------------- constants and loads ----------------
        ident = pp.tile([128, 128], BF16)
        make_identity(nc, ident[:])
        ident_f = pp.tile([128, 128], F32)
        make_identity(nc, ident_f[:])

        wq_s = pp.tile([D, D], F32)
        nc.sync.dma_start(wq_s[:], wq)
        wk_s = pp.tile([D, D], F32)
        nc.sync.dma_start(wk_s[:], wk)
        wv_s = pp.tile([D, D], F32)
        nc.sync.dma_start(wv_s[:], wv)
        bu_s = pp.tile([1, E], F32)
        nc.sync.dma_start(bu_s[:], bu[None, :])
        bu_rep = pp.tile([128, E], F32)
        nc.gpsimd.partition_broadcast(bu_rep[:], bu_s[0:1, :])

        # natural-layout bf16 copies of the big inputs (cast during DMA)
        xk_nat = pp.tile([128, KC, E], BF16)
        nc.gpsimd.dma_start(xk_nat[:], xk.rearrange("(j p) e -> p j e", p=128))
        xq_nat = pp.tile([128, QC, E], BF16)
        nc.gpsimd.dma_start(xq_nat[:], xq.rearrange("(j p) e -> p j e", p=128))
        wu_nat = pp.tile([128, EC, E], BF16)
        nc.gpsimd.dma_start(wu_nat[:], wu.rearrange("(j p) e -> p j e", p=128))

        # values with 64 ones-columns per head appended:
        # xv1[p, j, h, 0:64] = Xv, xv1[p, j, h, 64:128] = 1.0
        xv1 = pp.tile([128, KC, H * 128], BF16)
        xv1_v = xv1[:].rearrange("p j (h c) -> p j h c", c=128)
        for j in range(KC):
            nc.gpsimd.dma_start(
                xv1_v[:, j, :, 0:D],
                xv[j * 128 : (j + 1) * 128, :].rearrange("p (h d) -> p h d", d=D),
            )
        nc.gpsimd.memset(xv1_v[:, :, :, D:128], 1.0)

        # ---------------- on-chip transposes (PE) ----------------
        xkT = pp.tile([128, EC, S], BF16)   # [e, k]
        for t in range(EC):
            tbk = ep.tile([128, 1024], BF16, tag="et", name=f"tbk{t}")
            for j in range(KC):
                nc.tensor.transpose(
                    tbk[:, j * 128 : (j + 1) * 128],
                    xk_nat[:, j, t * 128 : (t + 1) * 128],
                    ident[:],
                )
            nc.vector.tensor_copy(xkT[:, t, :], tbk[:])

        xqT = pp.tile([128, EC, Q], BF16)   # [e, q]
        for t in range(EC):
            tbq = ep.tile([128, Q], BF16, tag="et", name=f"tbq{t}")
            for j in range(QC):
                nc.tensor.transpose(
                    tbq[:, j * 128 : (j + 1) * 128],
                    xq_nat[:, j, t * 128 : (t + 1) * 128],
                    ident[:],
                )
            nc.vector.tensor_copy(xqT[:, t, :], tbq[:])

        wuT = pp.tile([128, EC, E], BF16)   # [e, e']
        for t in range(EC):
            tbw = ep.tile([128, 1024], BF16, tag="et", name=f"tbw{t}")
            for j in range(EC):
                nc.tensor.transpose(
                    tbw[:, j * 128 : (j + 1) * 128],
                    wu_nat[:, j, t * 128 : (t + 1) * 128],
                    ident[:],
                )
            nc.vector.tensor_copy(wuT[:, t, :], tbw[:])

        # ---------------- A = Wq.T @ Wk, blkdiag weights ----------------
        apsum = sp.tile([D, D], F32, tag="spt", name="apsum")
        nc.tensor.matmul(apsum[:], wq_s[:], wk_s[:])  # = wq.T @ wk
        blkA = pp.tile([128, 128], BF16)
        nc.vector.memset(blkA[:], 0.0)
        nc.vector.tensor_copy(blkA[0:D, 0:D], apsum[:])
        nc.vector.tensor_copy(blkA[D:128, D:128], apsum[:])

        wvt_ps = sp.tile([D, D], F32, tag="spt", name="wvt_ps")
        nc.tensor.transpose(wvt_ps[:], wv_s[:], ident_f[0:D, 0:D])
        blkWvT = pp.tile([128, 128], BF16)
        nc.vector.memset(blkWvT[:], 0.0)
        nc.vector.tensor_copy(blkWvT[0:D, 0:D], wvt_ps[:])
        nc.vector.tensor_copy(blkWvT[D:128, D:128], wvt_ps[:])

        # ---------------- Qp.T = blkdiag(A).T-chunks @ Xq.T ----------------
        qpT = pp.tile([128, EC, Q], BF16)   # [e', q]
        for t in range(EC):
            qpp = sp.tile([128, Q], F32, tag="spt", name=f"qpp{t}")
            nc.tensor.matmul(qpp[:], blkA[:], xqT[:, t, :])
            nc.vector.tensor_copy(qpT[:, t, :], qpp[:])

        # ---------------- main loop over head pairs ----------------
        oT = pp.tile([128, EC, Q], BF16)    # context.T  [e, q]
        facc = pp.tile([128, QC, E], F32)   # final accumulator (natural)

        for p in range(8):  # pair p = heads (2p, 2p+1)
            pts = {}
            for hh in range(2):
                h = 2 * p + hh
                b0 = hh * D
                for ji, chunks in enumerate(JOB_CHUNKS):
                    w = 512 * len(chunks)
                    et = ep.tile([128, w], F32, tag="et", name=f"et{h}_{ji}")
                    for ci, c in enumerate(chunks):
                        # E_h.T chunk [k=128, q=512] = Xk_h.T-chunk.T @ Qp_h.T
                        nc.tensor.matmul(
                            et[:, ci * 512 : (ci + 1) * 512],
                            xkT[b0 : b0 + D, p, c * 128 : (c + 1) * 128],
                            qpT[b0 : b0 + D, p, :],
                        )
                    pt = ptp.tile([128, w], BF16, tag="pt", name=f"pt{h}_{ji}")
                    nc.scalar.activation(pt[:], et[:], AF.Exp, scale=SCALE)
                    pts[(hh, ji)] = pt

            cns = cnp.tile([128, Q], BF16, tag="cnt", name=f"cn{p}")
            for hh in range(2):
                h = 2 * p + hh
                b0 = hh * D
                cpt = cp.tile([128, Q], F32, tag="cpt", name=f"cpt{h}")
                for c in range(KC):
                    ji, ci = CHUNK2JOB[c]
                    # rows 0:64 accumulate P @ Xv_h ; rows 64:128 accumulate
                    # the softmax denominator (ones columns), replicated.
                    nc.tensor.matmul(
                        cpt[:],
                        xv1_v[:, c, h, :],
                        pts[(hh, ji)][:, ci * 512 : (ci + 1) * 512],
                        start=(c == 0),
                        stop=(c == KC - 1),
                    )
                rec = rcp.tile([128, Q], F32, tag="rec", name=f"rec{h}")
                nc.vector.reciprocal(rec[D:128, :], cpt[D:128, :])
                nc.vector.tensor_tensor(
                    cns[b0 : b0 + D, :], cpt[0:D, :], rec[D:128, :], op=ALU.mult
                )

            # O_pair.T = blkdiag(Wv,Wv) @ Cn_pair.T
            opt_ = sp.tile([128, Q], F32, tag="spt", name=f"opt{p}")
            nc.tensor.matmul(opt_[:], blkWvT[:], cns[:])
            nc.vector.tensor_copy(oT[:, p, :], opt_[:])

            # final projection partials: facc[s,:] += O_pair.T(s-slice).T @ Wu.T
            for s in range(QC):
                for half in range(2):
                    fpt = sp.tile([128, 512], F32, tag="spt", name=f"fp{p}_{s}_{half}")
                    nc.tensor.matmul(
                        fpt[:],
                        oT[:, p, s * 128 : (s + 1) * 128],
                        wuT[:, p, half * 512 : (half + 1) * 512],
                    )
                    dst = facc[:, s, half * 512 : (half + 1) * 512]
                    if p == 0:
                        nc.vector.tensor_tensor(
                            dst, fpt[:], bu_rep[:, half * 512 : (half + 1) * 512],
                            op=ALU.add,
                        )
                    else:
                        nc.vector.tensor_tensor(dst, dst, fpt[:], op=ALU.add)

        # ---------------- store ----------------
        for s in range(QC):
            nc.sync.dma_start(out[s * 128 : (s + 1) * 128, :], facc[:, s, :])


def build():
    nc = bacc.Bacc("TRN2", target_bir_lowering=False, debug=False)
    xq = nc.dram_tensor("xq", [Q, E], F32, kind="ExternalInput").ap()
    xk = nc.dram_tensor("xk", [S, E], F32, kind="ExternalInput").ap()
    xv = nc.dram_tensor("xv", [S, E], F32, kind="ExternalInput").ap()
    wq = nc.dram_tensor("wq", [D, D], F32, kind="ExternalInput").ap()
    wk = nc.dram_tensor("wk", [D, D], F32, kind="ExternalInput").ap()
    wv = nc.dram_tensor("wv", [D, D], F32, kind="ExternalInput").ap()
    wu = nc.dram_tensor("wu", [E, E], F32, kind="ExternalInput").ap()
    bu = nc.dram_tensor("bu", [E], F32, kind="ExternalInput").ap()
    out = nc.dram_tensor("out", [Q, E], F32, kind="ExternalOutput").ap()

    with tile.TileContext(nc) as tc:
        _body(nc, tc, xq, xk, xv, wq, wk, wv, wu, bu, out)
    nc.compile()
    return nc


_NC_CACHE = []


def _get_nc():
    if not _NC_CACHE:
        _NC_CACHE.append(build())
    return _NC_CACHE[0]


def _in_maps(values, keys, query, Wk, Wq, Wv, Wu, bu):
    values = np.ascontiguousarray(np.asarray(values, dtype=np.float32))
    keys = np.ascontiguousarray(np.asarray(keys, dtype=np.float32))
    query = np.ascontiguousarray(np.asarray(query, dtype=np.float32))
    Wk = np.ascontiguousarray(np.asarray(Wk, dtype=np.float32))
    Wq = np.ascontiguousarray(np.asarray(Wq, dtype=np.float32))
    Wv = np.ascontiguousarray(np.asarray(Wv, dtype=np.float32))
    Wu = np.ascontiguousarray(np.asarray(Wu, dtype=np.float32))
    bu = np.ascontiguousarray(np.asarray(bu, dtype=np.float32))
    maps = []
    for c in range(8):
        n, qh = divmod(c, 2)
        maps.append(
            {
                "xq": np.ascontiguousarray(query[n, qh * Q : (qh + 1) * Q, :]),
                "xk": keys[n],
                "xv": values[n],
                "wq": Wq,
                "wk": Wk,
                "wv": Wv,
                "wu": Wu,
                "bu": bu,
            }
        )
    return maps


def _ensure_ntff_hook():
    """The agent image's antenv lacks axon_hooks; bass_utils imports it when
    trace=True.  Inject the module and install the boot's ctypes-based hook."""
    import sys as _sys
    import types as _types

    if "antenv.axon_hooks" in _sys.modules:
        return
    try:
        import antenv  # noqa: F401

        mod = _types.ModuleType("antenv.axon_hooks")
        mod._hook = None

        def set_axon_ntff_profile_hook(h):
            mod._hook = h

        def get_axon_ntff_profile_hook():
            return mod._hook

        mod.set_axon_ntff_profile_hook = set_axon_ntff_profile_hook
        mod.get_axon_ntff_profile_hook = get_axon_ntff_profile_hook
        _sys.modules["antenv.axon_hooks"] = mod
        import antenv as _ae

        _ae.axon_hooks = mod
        from trn_agent_boot.trn_boot import _ntff_profile_via_ctypes

        mod._hook = _ntff_profile_via_ctypes("/opt/axon/libaxon_pjrt.so")
    except Exception:
        pass


def run(values, keys, query, mask, Wk, Wq, Wv, Wu, bu, trace=False):
    """Returns (full_output [4,1024,1024] f32, BassKernelResults)."""
    if trace:
        _ensure_ntff_hook()
    nc = _get_nc()
    maps = _in_maps(values, keys, query, Wk, Wq, Wv, Wu, bu)
    res = run_bass_kernel_spmd(nc, maps, core_ids=list(range(8)), trace=trace)
    out = np.empty((4, S, E), dtype=np.float32)
    for c in range(8):
        n, qh = divmod(c, 2)
        out[n, qh * Q : (qh + 1) * Q, :] = res.results[c]["out"]
    return out, res


def kernel(values, keys, query, mask, Wk, Wq, Wv, Wu, bu):
    out, _ = run(values, keys, query, mask, Wk, Wq, Wv, Wu, bu, trace=False)
    return out
